# revision 1
# baseline (speedup 1.0000x reference)
"""Single-head attention on 8 TRN2 NeuronCores, data-parallel over batch.

Per core (one batch element b):
  x_b [2048, 768] f32 -> Q = x Wq, K = x Wk, V = x Wv (head 64)
  scores^T[k, q] = (K^T slice).T @ Q^T / 8 ; E = exp(scores) (no max-sub:
  |scores| <~ 2.5 so exp is safe); out = (E^T' PV with ones row) -> normalize.

Layout strategy (everything contracts over the partition dim):
  - x: SWDGE cast-DMA f32->bf16 into SBUF natural [seq, emb], then X-bar
    DMA-transpose into x^T [emb, seq] (bf16). No TensorE transposes and no
    psum->SBUF copies for x^T.
  - Q^T/K^T computed with duplicated weights [Wq|Wq] so both partition
    halves hold the same 64 rows -> 2x row-tiled score matmuls (K=64
    contraction in row groups 0-1 / 2-3, alternating by k-tile parity).
  - exp on ScalarE in [128, 1024] batches (2 psum banks) to amortize the
    ~352-cycle per-instruction overhead; 1/sqrt(64) folded into the
    activation's free scale.
  - PV uses lhsT = V' = [V, ones] (M=65): psum row 64 accumulates the
    softmax denominator for free.
  - U^T [65, q] tiles are PE-transposed back to natural [q, 65]; col 64's
    reciprocal normalizes via tensor_scalar_mul, then DMA out.
"""

import numpy as np

import concourse.bass as bass
import concourse.tile as tile
from concourse import bacc, mybir
from concourse.bass_utils import run_bass_kernel_spmd
from concourse.masks import make_identity

B, S, D, H = 8, 2048, 768, 64
P = 128
NT = S // P  # 16 seq tiles
NCH = D // P  # 6 emb chunks
QC = 512  # q-chunk width (one psum bank of f32)
NQ = S // QC  # 4 q chunks
N_CORES = 8
F32 = mybir.dt.float32
BF16 = mybir.dt.bfloat16
EXP = mybir.ActivationFunctionType.Exp
SCALE = float(1.0 / np.sqrt(H))


def build_kernel():
    nc = bacc.Bacc("TRN2", num_devices=N_CORES)
    x_ext = nc.declare_dram_parameter("x", [S, D], F32, isOutput=False)
    wk_ext = nc.declare_dram_parameter("Wk", [D, H], F32, isOutput=False)
    wq_ext = nc.declare_dram_parameter("Wq", [D, H], F32, isOutput=False)
    wv_ext = nc.declare_dram_parameter("Wv", [D, H], F32, isOutput=False)
    out_ext = nc.declare_dram_parameter("out", [S, H], F32, isOutput=True)

    with tile.TileContext(nc) as tc:
        _body(nc, tc, x_ext, wq_ext, wk_ext, wv_ext, out_ext)
    nc.compile()
    return nc


def _body(nc, tc, x_ext, wq_ext, wk_ext, wv_ext, out_ext):
    with (
        tc.tile_pool(name="singles", bufs=1) as singles,
        tc.tile_pool(name="xn", bufs=3) as xn_pool,
        tc.tile_pool(name="et", bufs=3) as et_pool,
        tc.tile_pool(name="fin", bufs=4) as fin_pool,
    ):
        ident = singles.tile([P, P], F32)
        make_identity(nc, ident)
        ident_bf = singles.tile([P, P], BF16, tag="ident_bf")
        make_identity(nc, ident_bf)

        # ---- weights: DMA f32, cast to bf16, duplicate Q/K across halves
        wq_st = singles.tile([P, NCH, H], F32, tag="wst_q")
        wk_st = singles.tile([P, NCH, H], F32, tag="wst_k")
        wv_st = singles.tile([P, NCH, H], F32, tag="wst_v")
        for w_st, w_ext in ((wq_st, wq_ext), (wk_st, wk_ext), (wv_st, wv_ext)):
            nc.sync.dma_start(
                out=w_st, in_=w_ext.rearrange("(c p) h -> p c h", p=P))
        wq2 = singles.tile([P, NCH, 2 * H], BF16, tag="wq2")
        wk2 = singles.tile([P, NCH, 2 * H], BF16, tag="wk2")
        wv_sb = singles.tile([P, NCH, H], BF16, tag="wv_sb")
        nc.vector.tensor_copy(wq2[:, :, 0:H], wq_st)
        nc.vector.tensor_copy(wq2[:, :, H:2 * H], wq_st)
        nc.vector.tensor_copy(wk2[:, :, 0:H], wk_st)
        nc.vector.tensor_copy(wk2[:, :, H:2 * H], wk_st)
        nc.vector.tensor_copy(wv_sb, wv_st)

        # x^T chunk-major: copies and matmul reads are contiguous
        xt_sb = singles.tile([P, NCH, NT, P], BF16, tag="xt_sb")
        qt2 = singles.tile([P, S], BF16, tag="qt2")  # Q^T in both halves
        kt2 = singles.tile([P, S], BF16, tag="kt2")  # K^T in both halves
        vt_sb = singles.tile([H, S], BF16, tag="vt_sb")  # V^T
        vp = singles.tile([P, NT, H + 1], BF16, tag="vp")  # V' = [V, 1]
        nc.vector.memset(vp[:, :, H:H + 1], 1.0)

        # ---- phase 2: cast-DMA in, DMA-transpose, Q/K projections per strip
        # (V projections are deferred into phase 3 as PE gap-filler.)
        with (
            tc.tile_pool(name="ps_s", bufs=2, space="PSUM") as psum_s,
            tc.tile_pool(name="ps_u", bufs=2, space="PSUM") as psum_u_pool,
            tc.tile_pool(name="ps_v", bufs=2, space="PSUM") as psum_v,
        ):
            psum_p = psum_s  # QK-proj tiles share the score pool slots
            # x comes in as plain f32 SWDGE DMAs (efficient Q7 descriptors;
            # no X-bar anywhere in this kernel - DMA-transpose mode switches
            # serialize the whole DMA subsystem under Tile and can corrupt).
            # TensorE transposes x per 128x128 tile; DVE copies cast to bf16.
            xn_tiles = [xn_pool.tile([P, D], BF16, name=f"xn_{st}",
                                     tag=f"xn_{st}", bufs=1)
                        for st in range(NT)]
            for st in range(NT):
                xf = xn_pool.tile([P, D], F32, name=f"xf_{st}", tag="xf",
                                  bufs=6)
                nc.gpsimd.dma_start(
                    out=xf, in_=x_ext[st * P:(st + 1) * P, :])
                nc.vector.tensor_copy(xn_tiles[st], xf)
            def emit_transpose_group(sc, c):
                tsl = slice(sc * 4, (sc + 1) * 4)
                pst = psum_v.tile([P, 4, P], BF16, tag="pv", name="pst")
                for t in range(4):
                    nc.tensor.transpose(
                        pst[:, t, :],
                        xn_tiles[sc * 4 + t][:, c * P:(c + 1) * P],
                        ident_bf)
                # strips 0/1 copy on the (still idle) scalar engine; later
                # strips must stay off it so exps aren't FIFO-blocked
                if sc < 2:
                    nc.scalar.copy(out=xt_sb[:, c, tsl, :], in_=pst)
                else:
                    nc.vector.tensor_copy(xt_sb[:, c, tsl, :], pst)

            def emit_qk_proj(sc):
                sl = slice(sc * QC, (sc + 1) * QC)
                tsl = slice(sc * 4, (sc + 1) * 4)
                psqk = psum_p.tile([P, 2, QC], F32, tag="ss", name="psqk")
                for c in range(NCH):
                    nc.tensor.matmul(psqk[:, 0, :], wq2[:, c, :],
                                     xt_sb[:, c, tsl, :],
                                     start=(c == 0), stop=(c == NCH - 1))
                for c in range(NCH):
                    nc.tensor.matmul(psqk[:, 1, :], wk2[:, c, :],
                                     xt_sb[:, c, tsl, :],
                                     start=(c == 0), stop=(c == NCH - 1))
                nc.vector.tensor_copy(qt2[:, sl], psqk[:, 0, :])
                nc.vector.tensor_copy(kt2[:, sl], psqk[:, 1, :])

            def emit_v_proj(sc):
                # V^T projection for this strip (M=64, wide N), then
                # TensorE-transpose V^T back to natural layout inside V'
                sl = slice(sc * QC, (sc + 1) * QC)
                tsl = slice(sc * 4, (sc + 1) * 4)
                psvt = psum_v.tile([H, QC], F32, tag="pv", name="psvt")
                for c in range(NCH):
                    nc.tensor.matmul(psvt, wv_sb[:, c, :],
                                     xt_sb[:, c, tsl, :],
                                     start=(c == 0), stop=(c == NCH - 1))
                nc.vector.tensor_copy(vt_sb[:, sl], psvt)
                psvn = psum_v.tile([P, 4, H], BF16, tag="pv", name="psvn")
                for t in range(4):
                    nc.tensor.transpose(
                        psvn[:, t, :],
                        vt_sb[:, sc * QC + t * P:sc * QC + (t + 1) * P],
                        ident_bf[:H, :H])
                nc.vector.tensor_copy(vp[:, tsl, 0:H], psvn)

            def emit_strip(sc):
                for c in range(NCH):
                    emit_transpose_group(sc, c)
                emit_qk_proj(sc)
                emit_v_proj(sc)

            # strips 0-1 fully first (the qh0 k-loop needs their Q^T), with
            # strips 2-3 interleaved INTO the qh0 k-loop below so the first
            # score matmul isn't FIFO-queued behind all of phase 2 on PE.
            emit_strip(0)
            emit_strip(1)
            for c in range(NCH):
                emit_transpose_group(2, c)
            for c in range(NCH):
                emit_transpose_group(3, c)
            filler = {4: [("qk", 2, None)], 6: [("v", 2, None)],
                      9: [("qk", 3, None)], 11: [("v", 3, None)]}

            # ---- phase 3: per q-half: scores^T -> exp -> PV accumulate
            for qh in range(2):
                psum_u = [psum_u_pool.tile([H + 1, QC], F32, tag="pu",
                                           name=f"psum_u{qh}_{j}")
                          for j in range(2)]
                for kt in range(NT):
                    if qh == 0:
                        for kind, sc, c in filler.get(kt, []):
                            if kind == "tg":
                                emit_transpose_group(sc, c)
                            elif kind == "qk":
                                emit_qk_proj(sc)
                            else:
                                emit_v_proj(sc)
                    ksl = slice(kt * P, (kt + 1) * P)
                    et = et_pool.tile([P, 2 * QC], BF16, name="et")
                    ps = psum_s.tile([P, 2, QC], F32, tag="ss", name="ps")
                    for j in range(2):
                        lo = j * H  # row groups 0-1 / 2-3 run concurrently
                        qc = qh * 2 + j
                        nc.tensor.matmul(
                            ps[:, j, :], kt2[lo:lo + H, ksl],
                            qt2[lo:lo + H, qc * QC:(qc + 1) * QC],
                            start=True, stop=True)
                    nc.scalar.activation(
                        et.rearrange("p (a b) -> p a b", b=QC),
                        ps, EXP, scale=SCALE)
                    for j in range(2):
                        nc.tensor.matmul(
                            psum_u[j], vp[:, kt, :],
                            et[:, j * QC:(j + 1) * QC],
                            start=(kt == 0), stop=(kt == NT - 1))

                # ---- phase 4 (per half): transpose U^T, normalize, DMA out
                for j in range(2):
                    qc = qh * 2 + j
                    ut = fin_pool.tile([H + 1, QC], F32, tag="ut", name="ut")
                    nc.vector.tensor_copy(ut, psum_u[j])
                    for t in range(4):
                        qt = qc * 4 + t
                        pso = psum_v.tile([P, H + 1], F32, tag="pv",
                                          name="pso")
                        nc.tensor.transpose(
                            pso, ut[:, t * P:(t + 1) * P],
                            ident[:H + 1, :H + 1])
                        rcp = fin_pool.tile([P, 1], F32, tag="rcp",
                                            name="rcp")
                        nc.vector.reciprocal(rcp, pso[:, H:H + 1])
                        ot = fin_pool.tile([P, H], F32, tag="ot", name="ot")
                        nc.vector.tensor_scalar_mul(ot, pso[:, 0:H], rcp)
                        nc.sync.dma_start(
                            out=out_ext[qt * P:(qt + 1) * P, :], in_=ot)


_cached_nc = None


def kernel(**inputs):
    global _cached_nc
    x = np.ascontiguousarray(inputs["x"], dtype=np.float32)
    wk = np.ascontiguousarray(inputs["Wk"], dtype=np.float32)
    wq = np.ascontiguousarray(inputs["Wq"], dtype=np.float32)
    wv = np.ascontiguousarray(inputs["Wv"], dtype=np.float32)
    assert x.shape == (B, S, D)

    if _cached_nc is None:
        _cached_nc = build_kernel()
    nc = _cached_nc

    in_maps = [{"x": x[b], "Wk": wk, "Wq": wq, "Wv": wv} for b in range(B)]
    res = run_bass_kernel_spmd(nc, in_maps, core_ids=list(range(N_CORES)))
    return np.stack([res.results[i]["out"] for i in range(N_CORES)], axis=0)



# revision 5
# speedup vs baseline: 1.2624x; 1.2624x over previous
"""Single-head attention on 8 TRN2 NeuronCores, data-parallel over batch.

Per core (one batch element b):
  x_b [2048, 768] f32 -> Q = x Wq, K = x Wk, V = x Wv (head 64)
  scores^T[k, q] = K^T-slice.T @ Q^T / 8 ; E = exp(scores) (no max-sub:
  |scores| small); U = [V,1]^T-weighted sums of E give out + denominator.

v2 layout/schedule (vs v1):
  - x arrives via SWDGE cast-DMA f32->bf16 (no DVE casts at all).
  - Projections pack A=[Wq|Wk] and B=[Wv|Wq]; K^T is duplicated into both
    partition halves with one PE matmul against [I64|I64] so the score
    matmuls run as concurrent 2-way row-tiled pairs.
  - exp on ScalarE, one [128, 1024] ACTIVATE per k-tile straight from
    PSUM; ScalarE also evacuates strips 0-1 x^T while it is otherwise
    idle during the DMA ramp.
  - PV uses lhsT = [V, ones] (M=65); psum row 64 is the softmax
    denominator.
  - Strips 2-3 of phase 2 and the qh0 output tail are interleaved into
    the kt-loops' PE slack so the PE never idles long enough to
    re-throttle (HAM), and nothing serializes behind the input DMA.
"""

import numpy as np

import concourse.bass as bass
import concourse.tile as tile
from concourse import bacc, mybir
from concourse.bass_utils import run_bass_kernel_spmd
from concourse.masks import make_identity

B, S, D, H = 8, 2048, 768, 64
P = 128
NT = S // P  # 16 seq tiles
NCH = D // P  # 6 emb chunks
QC = 512
N_CORES = 8
F32 = mybir.dt.float32
BF16 = mybir.dt.bfloat16
EXP = mybir.ActivationFunctionType.Exp
SCALE = float(1.0 / np.sqrt(H))


def build_kernel():
    nc = bacc.Bacc("TRN2", num_devices=N_CORES)
    x_ext = nc.declare_dram_parameter("x", [S, D], F32, isOutput=False)
    wk_ext = nc.declare_dram_parameter("Wk", [D, H], F32, isOutput=False)
    wq_ext = nc.declare_dram_parameter("Wq", [D, H], F32, isOutput=False)
    wv_ext = nc.declare_dram_parameter("Wv", [D, H], F32, isOutput=False)
    out_ext = nc.declare_dram_parameter("out", [S, H], F32, isOutput=True)

    with tile.TileContext(nc) as tc:
        _body(nc, tc, x_ext, wq_ext, wk_ext, wv_ext, out_ext)
    nc.compile()
    return nc


def _body(nc, tc, x_ext, wq_ext, wk_ext, wv_ext, out_ext):
    with (
        tc.tile_pool(name="singles", bufs=1) as singles,
        tc.tile_pool(name="xn", bufs=3) as xn_pool,
        tc.tile_pool(name="et", bufs=3) as et_pool,
        tc.tile_pool(name="fin", bufs=4) as fin_pool,
        tc.tile_pool(name="ps", bufs=3, space="PSUM") as ps_pool,
        tc.tile_pool(name="uu", bufs=2, space="PSUM") as u_pool,
    ):
        ident = singles.tile([P, P], F32)
        make_identity(nc, ident)
        ident_bf = singles.tile([P, P], BF16, tag="ident_bf")
        make_identity(nc, ident_bf)
        # [I64|I64] in partitions 64-127: K^T-duplication stationary.
        dupI = singles.tile([P, P], BF16, tag="dupI")
        nc.vector.tensor_copy(dupI[64:P, 0:64], ident_bf[64:P, 64:P])
        nc.vector.tensor_copy(dupI[64:P, 64:P], ident_bf[64:P, 64:P])

        # warm the exp table set while everything else is still loading
        dummy = singles.tile([P, 8], BF16, tag="dummy")
        nc.scalar.activation(dummy, ident_bf[:, 0:8], EXP, scale=SCALE)

        # ---- weights: DMA f32, pack A=[Wq|Wk], B=[Wv|Wq] in bf16
        wq_st = singles.tile([P, NCH, H], F32, tag="wst_q")
        wk_st = singles.tile([P, NCH, H], F32, tag="wst_k")
        wv_st = singles.tile([P, NCH, H], F32, tag="wst_v")
        for w_st, w_ext in ((wq_st, wq_ext), (wk_st, wk_ext), (wv_st, wv_ext)):
            nc.sync.dma_start(
                out=w_st, in_=w_ext.rearrange("(c p) h -> p c h", p=P))
        wA = singles.tile([P, NCH, P], BF16, tag="wA")
        wB = singles.tile([P, NCH, P], BF16, tag="wB")
        nc.vector.tensor_copy(wA[:, :, 0:H], wq_st)
        nc.vector.tensor_copy(wA[:, :, H:P], wk_st)
        nc.vector.tensor_copy(wB[:, :, 0:H], wv_st)
        nc.vector.tensor_copy(wB[:, :, H:P], wq_st)

        # ---- persistent SBUF state
        xt_sb = singles.tile([P, NCH, NT, P], BF16, tag="xt_sb")  # x^T
        qkt = singles.tile([P, S], BF16, tag="qkt")   # [Q^T; K^T]
        qvt = singles.tile([P, S], BF16, tag="qvt")   # [V^T; Q^T]
        ktd = singles.tile([P, S], BF16, tag="ktd")   # K^T both halves
        vp = singles.tile([P, NT, H + 1], BF16, tag="vp")  # V' = [V, 1]
        nc.vector.memset(vp[:, :, H:H + 1], 1.0)

        # ---- x: 16 cast-DMAs f32->bf16 (SWDGE), issued up front
        xn_tiles = [xn_pool.tile([P, D], BF16, name=f"xn_{st}",
                                 tag=f"xn_{st}", bufs=1)
                    for st in range(NT)]
        for st in range(NT):
            nc.gpsimd.dma_start(
                out=xn_tiles[st], in_=x_ext[st * P:(st + 1) * P, :])

        # ---- phase-2 units (per strip sc of 4 seq tiles)
        def emit_trans(sc, c):
            tsl = slice(sc * 4, (sc + 1) * 4)
            pst = ps_pool.tile([P, 4, P], BF16, tag="ss", name="pst")
            for t in range(4):
                nc.tensor.transpose(
                    pst[:, t, :],
                    xn_tiles[sc * 4 + t][:, c * P:(c + 1) * P],
                    ident_bf)
            # strips 0-1 evac on the (still idle) scalar engine
            if sc < 2:
                nc.scalar.copy(out=xt_sb[:, c, tsl, :], in_=pst)
            else:
                nc.vector.tensor_copy(xt_sb[:, c, tsl, :], pst)

        def emit_projA(sc):
            sl = slice(sc * QC, (sc + 1) * QC)
            tsl = slice(sc * 4, (sc + 1) * 4)
            psA = ps_pool.tile([P, QC], F32, tag="ss", name="psA")
            for c in range(NCH):
                nc.tensor.matmul(psA, wA[:, c, :], xt_sb[:, c, tsl, :],
                                 start=(c == 0), stop=(c == NCH - 1))
            nc.vector.tensor_copy(qkt[:, sl], psA)

        def emit_projB(sc):
            sl = slice(sc * QC, (sc + 1) * QC)
            tsl = slice(sc * 4, (sc + 1) * 4)
            psB = ps_pool.tile([P, QC], F32, tag="ss", name="psB")
            for c in range(NCH):
                nc.tensor.matmul(psB, wB[:, c, :], xt_sb[:, c, tsl, :],
                                 start=(c == 0), stop=(c == NCH - 1))
            nc.vector.tensor_copy(qvt[:, sl], psB)

        def emit_kdup(sc):
            sl = slice(sc * QC, (sc + 1) * QC)
            psK = ps_pool.tile([P, QC], F32, tag="ss", name="psK")
            nc.tensor.matmul(psK, dupI[64:P, :], qkt[64:P, sl],
                             start=True, stop=True)
            nc.vector.tensor_copy(ktd[:, sl], psK)

        def emit_vtrans(sc):
            psv = ps_pool.tile([P, 4, H], BF16, tag="ss", name="psv")
            for t in range(4):
                off = sc * QC + t * P
                nc.tensor.transpose(
                    psv[:, t, :], qvt[0:H, off:off + P], ident_bf[:H, :H])
            nc.vector.tensor_copy(vp[:, sc * 4:(sc + 1) * 4, 0:H], psv)

        def emit_strip(sc):
            for c in range(NCH):
                emit_trans(sc, c)
            emit_projA(sc)
            emit_projB(sc)
            emit_kdup(sc)
            emit_vtrans(sc)

        emit_strip(0)
        emit_strip(1)

        # ---- output tail for one 128-row q tile
        ut_tiles = {}

        def emit_out(qt):
            ut = ut_tiles[qt // 4]
            pso = ps_pool.tile([P, H + 1], F32, tag="ss", name="pso")
            nc.tensor.transpose(
                pso, ut[:, (qt % 4) * P:(qt % 4 + 1) * P],
                ident[:H + 1, :H + 1])
            rcp = fin_pool.tile([P, 1], F32, tag="rcp", name="rcp")
            nc.vector.reciprocal(rcp, pso[:, H:H + 1])
            ot = fin_pool.tile([P, H], F32, tag="ot", name="ot")
            nc.vector.tensor_scalar_mul(ot, pso[:, 0:H], rcp)
            nc.sync.dma_start(out=out_ext[qt * P:(qt + 1) * P, :], in_=ot)

        # strips 2-3 interleave into qh0's kt loop; qh0's outs into qh1's
        filler = {
            (0, 1): [("tr", 2, 0), ("tr", 2, 1)],
            (0, 2): [("tr", 2, 2), ("tr", 2, 3)],
            (0, 3): [("tr", 2, 4), ("tr", 2, 5)],
            (0, 4): [("A", 2, 0), ("B", 2, 0)],
            (0, 5): [("kd", 2, 0), ("vt", 2, 0)],
            (0, 7): [("tr", 3, 0), ("tr", 3, 1)],
            (0, 8): [("tr", 3, 2), ("tr", 3, 3)],
            (0, 9): [("tr", 3, 4), ("tr", 3, 5)],
            (0, 10): [("A", 3, 0), ("B", 3, 0)],
            (0, 11): [("kd", 3, 0), ("vt", 3, 0)],
            (1, 0): [("out", 0, 0)],
            (1, 1): [("out", 1, 0)],
            (1, 2): [("out", 2, 0)],
            (1, 3): [("out", 3, 0)],
            (1, 4): [("out", 4, 0)],
            (1, 5): [("out", 5, 0)],
            (1, 6): [("out", 6, 0)],
            (1, 7): [("out", 7, 0)],
        }

        def run_filler(qh, kt):
            for kind, a1, a2 in filler.get((qh, kt), []):
                if kind == "tr":
                    emit_trans(a1, a2)
                elif kind == "A":
                    emit_projA(a1)
                elif kind == "B":
                    emit_projB(a1)
                elif kind == "kd":
                    emit_kdup(a1)
                elif kind == "vt":
                    emit_vtrans(a1)
                elif kind == "out":
                    emit_out(a1)

        # ---- main attention loops: per q-half, 16 k-tiles
        for qh in range(2):
            q0 = qh * 1024
            U = [u_pool.tile([H + 1, QC], F32, tag="pu",
                             name=f"U{qh}_{j}") for j in range(2)]
            for kt in range(NT):
                ksl = slice(kt * P, (kt + 1) * P)
                ss = ps_pool.tile([P, 2, QC], F32, tag="ss", name="ss")
                nc.tensor.matmul(
                    ss[:, 0, :], ktd[0:H, ksl],
                    qkt[0:H, q0:q0 + QC], start=True, stop=True)
                nc.tensor.matmul(
                    ss[:, 1, :], ktd[H:P, ksl],
                    qvt[H:P, q0 + QC:q0 + 2 * QC], start=True, stop=True)
                et = et_pool.tile([P, 2, QC], BF16, name="et")
                nc.scalar.activation(et, ss, EXP, scale=SCALE)
                for j in range(2):
                    nc.tensor.matmul(
                        U[j], vp[:, kt, :], et[:, j, :],
                        start=(kt == 0), stop=(kt == NT - 1))
                run_filler(qh, kt)

            # evacuate U and schedule this half's output tiles
            for j in range(2):
                ut = fin_pool.tile([H + 1, QC], F32, tag=f"ut{j}",
                                   name=f"ut{qh}_{j}")
                nc.vector.tensor_copy(ut, U[j])
                ut_tiles[qh * 2 + j] = ut

        # qh1's own output tiles (qh0's ran as qh1 fillers)
        for qt in range(8, 16):
            emit_out(qt)


_cached_nc = None


def kernel(**inputs):
    global _cached_nc
    x = np.ascontiguousarray(inputs["x"], dtype=np.float32)
    wk = np.ascontiguousarray(inputs["Wk"], dtype=np.float32)
    wq = np.ascontiguousarray(inputs["Wq"], dtype=np.float32)
    wv = np.ascontiguousarray(inputs["Wv"], dtype=np.float32)
    assert x.shape == (B, S, D)

    if _cached_nc is None:
        _cached_nc = build_kernel()
    nc = _cached_nc

    in_maps = [{"x": x[b], "Wk": wk, "Wq": wq, "Wv": wv} for b in range(B)]
    res = run_bass_kernel_spmd(nc, in_maps, core_ids=list(range(N_CORES)))
    return np.stack([res.results[i]["out"] for i in range(N_CORES)], axis=0)


# revision 8
# speedup vs baseline: 1.3349x; 1.0575x over previous
"""Single-head attention on 8 TRN2 NeuronCores, data-parallel over batch.

Per core (one batch element b):
  x_b [2048, 768] f32 -> Q = x Wq, K = x Wk, V = x Wv (head 64)
  scores^T[k, q] = K^T-slice.T @ Q^T / 8 ; E = exp(scores) (no max-sub:
  |scores| small); U = [V,1]^T-weighted sums of E give out + denominator.

v3 layout/schedule:
  - x arrives via SWDGE cast-DMA f32->bf16, issued at the very head of
    the gpsimd queue so HBM streaming starts immediately.
  - Projections pack A=[Wq|Wk] and B=[Wv|Wq]; K^T is duplicated into both
    partition halves with one PE matmul against [I64|I64] so the score
    matmuls run as concurrent 2-way row-tiled pairs.
  - exp on ScalarE, one [128, 1024] ACTIVATE per k-tile straight from
    PSUM. The kt loop is software-pipelined in emission order
    (scores(kt) -> exp(kt) -> PV(kt-1) -> fillers) so exp never queues
    behind PV or phase-2 work on the PE FIFO.
  - Separate PSUM pools: scores double-buffer (4 banks) + phase-2 (2) +
    PV accumulators (2) = exactly 8 banks.
  - PV uses lhsT = [V, ones] (M=65); psum row 64 is the softmax
    denominator. Outputs collect in one SBUF buffer per q-half and leave
    as a single DMA each.
"""

import numpy as np

import concourse.bass as bass
import concourse.tile as tile
from concourse import bacc, mybir
from concourse.bass_utils import run_bass_kernel_spmd
from concourse.masks import make_identity

B, S, D, H = 8, 2048, 768, 64
P = 128
NT = S // P  # 16 seq tiles
NCH = D // P  # 6 emb chunks
QC = 512
N_CORES = 8
F32 = mybir.dt.float32
BF16 = mybir.dt.bfloat16
EXP = mybir.ActivationFunctionType.Exp
SCALE = float(1.0 / np.sqrt(H))


def build_kernel():
    nc = bacc.Bacc("TRN2", num_devices=N_CORES)
    x_ext = nc.declare_dram_parameter("x", [S, D], F32, isOutput=False)
    wk_ext = nc.declare_dram_parameter("Wk", [D, H], F32, isOutput=False)
    wq_ext = nc.declare_dram_parameter("Wq", [D, H], F32, isOutput=False)
    wv_ext = nc.declare_dram_parameter("Wv", [D, H], F32, isOutput=False)
    out_ext = nc.declare_dram_parameter("out", [S, H], F32, isOutput=True)

    with tile.TileContext(nc) as tc:
        _body(nc, tc, x_ext, wq_ext, wk_ext, wv_ext, out_ext)
    nc.compile()
    return nc


def _body(nc, tc, x_ext, wq_ext, wk_ext, wv_ext, out_ext):
    with (
        tc.tile_pool(name="singles", bufs=1) as singles,
        tc.tile_pool(name="xn", bufs=3) as xn_pool,
        tc.tile_pool(name="et", bufs=3) as et_pool,
        tc.tile_pool(name="fin", bufs=4) as fin_pool,
        tc.tile_pool(name="ph2", bufs=2, space="PSUM") as ph2,
        tc.tile_pool(name="ss", bufs=2, space="PSUM") as ss_pool,
        tc.tile_pool(name="uu", bufs=2, space="PSUM") as u_pool,
    ):
        # ---- x cast-DMAs head the gpsimd queue; identities slot in
        # after strip 0+2 tiles so the first transposes aren't blocked.
        xn_tiles = [xn_pool.tile([P, D], BF16, name=f"xn_{st}",
                                 tag=f"xn_{st}", bufs=1)
                    for st in range(NT)]

        def dma_x(st):
            nc.gpsimd.dma_start(
                out=xn_tiles[st], in_=x_ext[st * P:(st + 1) * P, :])

        for st in range(6):
            dma_x(st)

        ident = singles.tile([P, P], F32)
        make_identity(nc, ident)
        ident_bf = singles.tile([P, P], BF16, tag="ident_bf")
        make_identity(nc, ident_bf)
        # [I64|I64] in partitions 64-127: K^T-duplication stationary.
        dupI = singles.tile([P, P], BF16, tag="dupI")
        nc.vector.tensor_copy(dupI[64:P, 0:64], ident_bf[64:P, 64:P])
        nc.vector.tensor_copy(dupI[64:P, 64:P], ident_bf[64:P, 64:P])
        vp = singles.tile([P, NT, H + 1], BF16, tag="vp")  # V' = [V, 1]
        nc.vector.memset(vp[:, :, H:H + 1], 1.0)

        # warm the exp table set while everything else is still loading
        dummy = singles.tile([P, 8], BF16, tag="dummy")
        nc.scalar.activation(dummy, ident_bf[:, 0:8], EXP, scale=SCALE)

        for st in range(6, NT):
            dma_x(st)

        # ---- weights: DMA f32, pack A=[Wq|Wk], B=[Wv|Wq] in bf16
        wq_st = singles.tile([P, NCH, H], F32, tag="wst_q")
        wk_st = singles.tile([P, NCH, H], F32, tag="wst_k")
        wv_st = singles.tile([P, NCH, H], F32, tag="wst_v")
        for w_st, w_ext in ((wq_st, wq_ext), (wk_st, wk_ext), (wv_st, wv_ext)):
            nc.sync.dma_start(
                out=w_st, in_=w_ext.rearrange("(c p) h -> p c h", p=P))
        wA = singles.tile([P, NCH, P], BF16, tag="wA")
        wB = singles.tile([P, NCH, P], BF16, tag="wB")
        nc.vector.tensor_copy(wA[:, :, 0:H], wq_st)
        nc.vector.tensor_copy(wA[:, :, H:P], wk_st)
        nc.vector.tensor_copy(wB[:, :, 0:H], wv_st)
        nc.vector.tensor_copy(wB[:, :, H:P], wq_st)

        # ---- persistent SBUF state
        xt_sb = singles.tile([P, NCH, NT, P], BF16, tag="xt_sb")  # x^T
        qkt = singles.tile([P, S], BF16, tag="qkt")   # [Q^T; K^T]
        qvt = singles.tile([P, S], BF16, tag="qvt")   # [V^T; Q^T]
        ktd = singles.tile([P, S], BF16, tag="ktd")   # K^T both halves

        # ---- phase-2 units (per strip sc of 4 seq tiles)
        def emit_trans(sc, c):
            tsl = slice(sc * 4, (sc + 1) * 4)
            pst = ph2.tile([P, 4, P], BF16, tag="ph2", name="pst")
            for t in range(4):
                nc.tensor.transpose(
                    pst[:, t, :],
                    xn_tiles[sc * 4 + t][:, c * P:(c + 1) * P],
                    ident_bf)
            # strips 0-1 evac on the (still idle) scalar engine
            if sc < 2:
                nc.scalar.copy(out=xt_sb[:, c, tsl, :], in_=pst)
            else:
                nc.vector.tensor_copy(xt_sb[:, c, tsl, :], pst)

        def emit_projA(sc):
            sl = slice(sc * QC, (sc + 1) * QC)
            tsl = slice(sc * 4, (sc + 1) * 4)
            psA = ph2.tile([P, QC], F32, tag="ph2", name="psA")
            for c in range(NCH):
                nc.tensor.matmul(psA, wA[:, c, :], xt_sb[:, c, tsl, :],
                                 start=(c == 0), stop=(c == NCH - 1))
            nc.vector.tensor_copy(qkt[:, sl], psA)

        def emit_projB(sc):
            sl = slice(sc * QC, (sc + 1) * QC)
            tsl = slice(sc * 4, (sc + 1) * 4)
            psB = ph2.tile([P, QC], F32, tag="ph2", name="psB")
            for c in range(NCH):
                nc.tensor.matmul(psB, wB[:, c, :], xt_sb[:, c, tsl, :],
                                 start=(c == 0), stop=(c == NCH - 1))
            nc.vector.tensor_copy(qvt[:, sl], psB)

        def emit_kdup(sc):
            sl = slice(sc * QC, (sc + 1) * QC)
            psK = ph2.tile([P, QC], F32, tag="ph2", name="psK")
            nc.tensor.matmul(psK, dupI[64:P, :], qkt[64:P, sl],
                             start=True, stop=True)
            nc.vector.tensor_copy(ktd[:, sl], psK)

        def emit_vtrans(sc):
            psv = ph2.tile([P, 4, H], BF16, tag="ph2", name="psv")
            for t in range(4):
                off = sc * QC + t * P
                nc.tensor.transpose(
                    psv[:, t, :], qvt[0:H, off:off + P], ident_bf[:H, :H])
            nc.vector.tensor_copy(vp[:, sc * 4:(sc + 1) * 4, 0:H], psv)

        def emit_strip(sc):
            for c in range(NCH):
                emit_trans(sc, c)
            emit_projA(sc)
            emit_projB(sc)
            emit_kdup(sc)
            emit_vtrans(sc)

        emit_strip(0)
        emit_strip(1)

        # ---- output tail for one 128-row q tile; batched DMA per q-half
        ut_tiles = {}
        ob_tiles = {}

        def emit_out(qt):
            ut = ut_tiles[qt // 4]
            ob = ob_tiles[qt // 8]
            pso = ph2.tile([P, H + 1], F32, tag="ph2", name="pso")
            nc.tensor.transpose(
                pso, ut[:, (qt % 4) * P:(qt % 4 + 1) * P],
                ident[:H + 1, :H + 1])
            rcp = fin_pool.tile([P, 1], F32, tag="rcp", name="rcp")
            nc.vector.reciprocal(rcp, pso[:, H:H + 1])
            nc.vector.tensor_scalar_mul(ob[:, qt % 8, :], pso[:, 0:H], rcp)
            if qt % 8 == 7:
                half = qt // 8
                nc.sync.dma_start(
                    out=out_ext[half * 1024:(half + 1) * 1024, :].rearrange(
                        "(t p) h -> p t h", p=P),
                    in_=ob)

        # strips 2-3 interleave into qh0's kt loop; qh0's outs into qh1's
        # filler at key (qh, k) is emitted in pipeline group k+1; sc(kt)
        # precedes fill(kt-1) on the PE FIFO, so a unit feeding sc(kt)/PV(kt)
        # must sit at key <= kt-2 / kt-1 respectively.
        filler = {
            (0, 1): [("tr", 2, 0), ("tr", 2, 1)],
            (0, 2): [("tr", 2, 2), ("tr", 2, 3)],
            (0, 3): [("tr", 2, 4), ("tr", 2, 5)],
            (0, 4): [("A", 2, 0)],
            (0, 5): [("kd", 2, 0), ("B", 2, 0), ("tr", 3, 0)],
            (0, 6): [("vt", 2, 0), ("tr", 3, 1)],
            (0, 7): [("tr", 3, 2), ("tr", 3, 3)],
            (0, 8): [("tr", 3, 4), ("tr", 3, 5)],
            (0, 9): [("A", 3, 0)],
            (0, 10): [("kd", 3, 0), ("B", 3, 0)],
            (0, 11): [("vt", 3, 0)],
            (1, 0): [("out", 0, 0)],
            (1, 1): [("out", 1, 0)],
            (1, 2): [("out", 2, 0)],
            (1, 3): [("out", 3, 0)],
            (1, 4): [("out", 4, 0)],
            (1, 5): [("out", 5, 0)],
            (1, 6): [("out", 6, 0)],
            (1, 7): [("out", 7, 0)],
        }

        def run_filler(qh, kt):
            for kind, a1, a2 in filler.get((qh, kt), []):
                if kind == "tr":
                    emit_trans(a1, a2)
                elif kind == "A":
                    emit_projA(a1)
                elif kind == "B":
                    emit_projB(a1)
                elif kind == "kd":
                    emit_kdup(a1)
                elif kind == "vt":
                    emit_vtrans(a1)
                elif kind == "out":
                    emit_out(a1)

        # ---- main attention loops: per q-half, 16 k-tiles,
        # software-pipelined emission: sc(kt), exp(kt), PV(kt-1), fill(kt-1)
        for qh in range(2):
            q0 = qh * 1024
            U = [u_pool.tile([H + 1, QC], F32, tag="pu",
                             name=f"U{qh}_{j}") for j in range(2)]
            et_tiles = {}
            for kt in range(NT):
                ksl = slice(kt * P, (kt + 1) * P)
                ss = ss_pool.tile([P, 2, QC], F32, tag="ss", name="ss")
                nc.tensor.matmul(
                    ss[:, 0, :], ktd[0:H, ksl],
                    qkt[0:H, q0:q0 + QC], start=True, stop=True)
                nc.tensor.matmul(
                    ss[:, 1, :], ktd[H:P, ksl],
                    qvt[H:P, q0 + QC:q0 + 2 * QC], start=True, stop=True)
                et = et_pool.tile([P, 2, QC], BF16, name="et")
                nc.scalar.activation(et, ss, EXP, scale=SCALE)
                et_tiles[kt] = et
                if kt > 0:
                    for j in range(2):
                        nc.tensor.matmul(
                            U[j], vp[:, kt - 1, :], et_tiles[kt - 1][:, j, :],
                            start=(kt - 1 == 0), stop=False)
                    del et_tiles[kt - 1]
                    run_filler(qh, kt - 1)
            for j in range(2):
                nc.tensor.matmul(
                    U[j], vp[:, NT - 1, :], et_tiles[NT - 1][:, j, :],
                    start=False, stop=True)
            run_filler(qh, NT - 1)

            # evacuate U and stage this half's output buffer
            ob_tiles[qh] = fin_pool.tile([P, 8, H], F32, tag=f"ob{qh}",
                                         name=f"ob{qh}", bufs=1)
            for j in range(2):
                ut = fin_pool.tile([H + 1, QC], F32, tag=f"ut{j}",
                                   name=f"ut{qh}_{j}")
                nc.vector.tensor_copy(ut, U[j])
                ut_tiles[qh * 2 + j] = ut

        # qh1's own output tiles (qh0's ran as qh1 fillers)
        for qt in range(8, 16):
            emit_out(qt)


_cached_nc = None


def kernel(**inputs):
    global _cached_nc
    x = np.ascontiguousarray(inputs["x"], dtype=np.float32)
    wk = np.ascontiguousarray(inputs["Wk"], dtype=np.float32)
    wq = np.ascontiguousarray(inputs["Wq"], dtype=np.float32)
    wv = np.ascontiguousarray(inputs["Wv"], dtype=np.float32)
    assert x.shape == (B, S, D)

    if _cached_nc is None:
        _cached_nc = build_kernel()
    nc = _cached_nc

    in_maps = [{"x": x[b], "Wk": wk, "Wq": wq, "Wv": wv} for b in range(B)]
    res = run_bass_kernel_spmd(nc, in_maps, core_ids=list(range(N_CORES)))
    return np.stack([res.results[i]["out"] for i in range(N_CORES)], axis=0)
